# revision 3
# baseline (speedup 1.0000x reference)
"""Bass/Tile TRN2 kernel for nn_ExpressionAttentionLayer.

Math per batch b (B=8, G=2048, D=64):
    K_fused = concat([K_gene, K_expr], -1) @ WK_w.T + WK_b      # (G, D)
    Q_fused = concat([Q_gene, Q_expr], -1) @ WQ_w.T + WQ_b      # (G, D)
    A       = softmax(Q_fused @ K_fused.T / sqrt(D), axis=-1)
    out     = (A * M) @ V_expr                                   # (G, D)

Sharding: data-parallel over batch; core i handles batch i (B == n_cores == 8).
No collectives.

Per-core dataflow (v3):
  - All Q/K loads use the "(p s)" partition-contiguous DRAM layout (1KB+
    descriptors).  PE transpose-mode runs directly on fp32 (1 pass on
    cayman), so no pre-casts.
  - K side feeds both HWDGE rings first (kfT gates every logits matmul);
    its transposed chunks scatter to stride-16 column views to restore
    natural k order (k must line up with M's columns).
  - Q side keeps the chunk order: loop iteration qt computes the 128
    logical q rows {p*16 + qt}.  The M tile and the output tile for
    iteration qt use row-permuted DRAM views (M stays 8KB/partition
    descriptors), so no on-chip fixup is needed and iteration qt only
    depends on Q chunk qt -> the loop starts ~7us in.
  - M streams in as bf16 via SWDGE cast-DMA half-tiles on the gpsimd
    queue (its own ring; 16 MB of the ~19.4 MB total HBM read traffic).
  - Per q-tile, per 1024-col half h: logits psum(fp32) = Q_tile.T @
    K_fusedT (bf16); exp on ScalarE -> bf16 ex with fp32 row-sum accum
    (scale=1/sqrt(D) folded in; |logits| <~ 7 so no max-subtraction
    needed); em = ex * M_bf16 on VectorE (all-bf16 -> 2x DVE mode);
    PE-transpose em (bf16 psum); single DVE 2x copy to [k, q] sbuf
    tiles; accumulate out over k-tiles on PE; apply the softmax
    reciprocal on ScalarE while copying out of PSUM; DMA out on sync.
"""

from contextlib import ExitStack

import numpy as np

import concourse.bass as bass
import concourse.tile as tile
from concourse import bacc, mybir
from concourse.bass_utils import run_bass_kernel_spmd
from concourse.masks import make_identity

B, G, D = 8, 2048, 64
P = 128
NT = G // P  # 16 tiles of 128 rows
F32 = mybir.dt.float32
BF16 = mybir.dt.bfloat16
AF = mybir.ActivationFunctionType

N_CORES = 8


def _emit(ctx: ExitStack, tc: tile.TileContext, io: dict):
    nc = tc.nc

    singles = ctx.enter_context(tc.tile_pool(name="singles", bufs=1))
    ld = ctx.enter_context(tc.tile_pool(name="ld", bufs=4))

    # PSUM pools (8 banks total: ps_l 2x2 + ps_t 2x1 + ps_o 2x1 = 8)
    ps_l = ctx.enter_context(tc.tile_pool(name="ps_l", bufs=2, space="PSUM"))
    ps_t = ctx.enter_context(tc.tile_pool(name="ps_t", bufs=2, space="PSUM"))
    ps_o = ctx.enter_context(tc.tile_pool(name="ps_o", bufs=2, space="PSUM"))

    identity = singles.tile([P, P], F32)
    make_identity(nc, identity[:])
    identity_bf = singles.tile([P, P], BF16)
    nc.vector.tensor_copy(identity_bf[:], identity[:])

    # ---- M-tile SWDGE cast-DMA lookahead queue (bf16 in SBUF) ----
    # Row-permuted view: iteration qt covers logical q rows {p*16 + qt},
    # matching the Q-side chunk order (see module docstring).  8KB
    # contiguous per partition per half -> line-rate descriptors.
    mpool = ctx.enter_context(tc.tile_pool(name="mpool", bufs=5))
    m_r = io["M"].rearrange("(p s) k -> s p k", s=NT)
    mts = {}

    def issue_m(qt):
        if qt < NT:
            mt = mpool.tile([P, G], BF16, tag="m", name="m")
            for h in range(2):
                nc.gpsimd.dma_start(
                    mt[:, h * 1024 : (h + 1) * 1024],
                    m_r[qt, :, h * 1024 : (h + 1) * 1024],
                )
            mts[qt] = mt

    for _qt in range(4):
        issue_m(_qt)

    # ---- HAM warmup: ~3.5us of junk matmuls while the first DMAs land, so
    # the PE clock is at 2.4 GHz when real work starts.
    junk = singles.tile([P, 512], BF16, tag="junk")
    nc.vector.memset(junk[:], 0.0)
    for _ in range(8):
        psw = ps_o.tile([P, 512], F32, tag="ps_o", name="ps_warm")
        nc.tensor.matmul(psw[:], identity_bf[:], junk[:], start=True, stop=True)

    # ---- weights: WK_w/WQ_w are [D, 2D]; natural load, then PE-transpose the
    # two [64, 64] halves (base partition 0) and cast to bf16 lhsT tiles.
    wk_nat = singles.tile([D, 2 * D], F32, tag="wk_nat")
    wq_nat = singles.tile([D, 2 * D], F32, tag="wq_nat")
    nc.sync.dma_start(wk_nat[:], io["WK_w"][:, :])
    nc.scalar.dma_start(wq_nat[:], io["WQ_w"][:, :])
    wk_gTb = singles.tile([D, D], BF16, tag="wk_gTb")
    wk_eTb = singles.tile([D, D], BF16, tag="wk_eTb")
    wq_gTb = singles.tile([D, D], BF16, tag="wq_gTb")
    wq_eTb = singles.tile([D, D], BF16, tag="wq_eTb")
    for nat, dsts in ((wk_nat, (wk_gTb, wk_eTb)), (wq_nat, (wq_gTb, wq_eTb))):
        for h, dst in enumerate(dsts):
            psw = ps_o.tile([P, 512], F32, tag="ps_o", name="ps_w")
            nc.tensor.transpose(
                psw[:D, :D], nat[:, h * D : (h + 1) * D], identity[:D, :D]
            )
            nc.vector.tensor_copy(dst[:], psw[:D, :D])
    wkb = singles.tile([D, 1], F32, tag="wkb")
    wqb = singles.tile([D, 1], F32, tag="wqb")
    nc.sync.dma_start(wkb[:], io["WK_b"][:, None])
    nc.scalar.dma_start(wqb[:], io["WQ_b"][:, None])

    # ---- input loads: "(p s)" partition-contiguous layout, quartered.
    # K side split across BOTH HWDGE rings (it gates everything); Q side
    # follows on both rings; V last (first needed by the AV stage).
    bigs = {}
    for src_name, eng in (
        ("K_gene", nc.sync),
        ("K_expr", nc.scalar),
        ("Q_gene", nc.sync),
        ("Q_expr", nc.scalar),
    ):
        big = ld.tile([P, NT, D], F32, tag=f"ld_{src_name}", name=f"ld_{src_name}")
        r = io[src_name].rearrange("(p s) d -> p s d", s=NT)
        for q4 in range(4):
            eng.dma_start(big[:, 4 * q4 : 4 * q4 + 4, :], r[:, 4 * q4 : 4 * q4 + 4, :])
        bigs[src_name] = big

    # ---- V in [128, NT, D] (natural g on partitions; 256B descriptors but
    # only 512KB and not needed until the first AV), cast to bf16.
    v_sb = singles.tile([P, NT, D], F32, tag="v")
    nc.scalar.dma_start(v_sb[:], io["V_expr"].rearrange("(t p) d -> p t d", p=P))
    v_bf = singles.tile([P, NT, D], BF16, tag="v_bf")
    nc.vector.tensor_copy(v_bf[:], v_sb[:])

    # ---- transpose K/Q gene+expr into bf16 [D, G] (d on partitions) ----
    # Chunk s of the "(p s)" load transposes to columns {p*16 + s}.  K side
    # scatters through a stride-16 view to restore natural k order; Q side
    # keeps chunk order (the q permutation is absorbed by the M/out views).
    kgT = singles.tile([D, G], BF16, tag="kgT")
    keT = singles.tile([D, G], BF16, tag="keT")
    qgT = singles.tile([D, G], BF16, tag="qgT")
    qeT = singles.tile([D, G], BF16, tag="qeT")
    kfT = singles.tile([D, G], BF16, tag="kfT")
    qfT = singles.tile([D, G], BF16, tag="qfT")

    def emit_transposes(side, gT, eT, wgT, weT, b_sb, fT, j):
        for c, dstT, ceng in ((0, gT, nc.vector), (1, eT, nc.scalar)):
            big = bigs[f"{side}_gene" if c == 0 else f"{side}_expr"]
            ps = ps_l.tile([P, 1024], F32, tag="ps_l", name="ps_tr")[:D, :512]
            for i in range(4):
                s = 4 * j + i
                nc.tensor.transpose(
                    ps[:, i * P : (i + 1) * P], big[:, s, :], identity[:]
                )
            src = ps[:].rearrange("d (i p) -> d i p", i=4)
            if side == "K":
                # natural order: column p*16 + s <- chunk s position p
                dst = dstT[:].rearrange("d (p s) -> d s p", s=NT)[:, 4 * j : 4 * j + 4, :]
            else:
                # chunk order: chunk s occupies columns [s*128, (s+1)*128)
                dst = dstT[:, j * 512 : (j + 1) * 512].rearrange(
                    "d (i p) -> d i p", i=4
                )
            if c == 0:
                ceng.tensor_copy(dst, src)
            else:
                ceng.copy(dst, src)
        if side == "Q":
            # Q projections can run per block (block j = chunks 4j..4j+3)
            emit_proj(gT, eT, wgT, weT, b_sb, fT, j)

    def emit_proj(gT, eT, wgT, weT, b_sb, fT, j):
        psj = ps_o.tile([P, 512], F32, tag="ps_o", name="ps_pj")[:D, :]
        nc.tensor.matmul(
            psj[:], wgT[:], gT[:, j * 512 : (j + 1) * 512], start=True, stop=False
        )
        nc.tensor.matmul(
            psj[:], weT[:], eT[:, j * 512 : (j + 1) * 512], start=False, stop=True
        )
        nc.scalar.activation(
            fT[:, j * 512 : (j + 1) * 512], psj[:], AF.Identity, bias=b_sb[:, 0:1]
        )

    for j in range(4):
        emit_transposes("K", kgT, keT, wk_gTb, wk_eTb, wkb, kfT, j)
    for j in range(4):
        # K projections only after all K chunks scattered (stride-16 mix)
        emit_proj(kgT, keT, wk_gTb, wk_eTb, wkb, kfT, j)
    for j in range(4):
        emit_transposes("Q", qgT, qeT, wq_gTb, wq_eTb, wqb, qfT, j)

    # ---- main attention loop (fully per-q-tile pipelined) ----
    epool = ctx.enter_context(tc.tile_pool(name="epool", bufs=2))
    empool = ctx.enter_context(tc.tile_pool(name="empool", bufs=2))
    tpool = ctx.enter_context(tc.tile_pool(name="tpool", bufs=2))
    opool = ctx.enter_context(tc.tile_pool(name="opool", bufs=2))
    rspool = ctx.enter_context(tc.tile_pool(name="rspool", bufs=4))

    out_r = io["out"].rearrange("(p s) d -> s p d", s=NT)
    scale = 1.0 / np.sqrt(np.float32(D))

    # Per-qt state carried one step so the AV matmuls of qt-1 are emitted
    # between qt's logits and qt's transposes — PE chews on them while the
    # ScalarE/VectorE stages of qt run, instead of stalling at a group
    # barrier.
    pending = None  # (qt, emt, recip)

    def emit_av(pend):
        qt_p, emt_p, recip_p = pend
        # out[q, d] += expM^T_chunk.T @ V  (lhsT=emt chunk: 128 bf16 cols -> FWL)
        pso = ps_o.tile([P, 512], F32, tag="ps_o", name="ps_av")[:, :D]
        for kt in range(NT):
            nc.tensor.matmul(
                pso[:],
                emt_p[:, kt, :],
                v_bf[:, kt, :],
                start=(kt == 0),
                stop=(kt == NT - 1),
            )
        ob = opool.tile([P, D], F32, tag="ob")
        # apply softmax denominator while copying out of PSUM
        nc.scalar.activation(ob[:], pso[:], AF.Copy, bias=0.0, scale=recip_p[:, 0:1])
        nc.sync.dma_start(out_r[qt_p], ob[:])

    for qt in range(NT):
        mt = mts.pop(qt)
        issue_m(qt + 4)

        ex = epool.tile([P, G], BF16, tag="ex")
        em = empool.tile([P, G], BF16, tag="em")
        emt = tpool.tile([P, NT, P], BF16, tag="emt")  # expM^T tiles [k, q]
        rs = [rspool.tile([P, 1], F32, tag=f"rs{h}", name=f"rs{h}") for h in range(2)]

        for h in range(2):
            # logits in a [128, 1024] psum tile (2 banks)
            psl = ps_l.tile([P, 1024], F32, tag="ps_l")
            for n in range(2):
                nc.tensor.matmul(
                    psl[:, n * 512 : (n + 1) * 512],
                    qfT[:, qt * P : (qt + 1) * P],
                    kfT[:, (2 * h + n) * 512 : (2 * h + n + 1) * 512],
                    start=True,
                    stop=True,
                )
            # exp -> bf16 with fp32 row-sum accumulation
            nc.scalar.activation(
                ex[:, h * 1024 : (h + 1) * 1024],
                psl[:],
                AF.Exp,
                scale=float(scale),
                accum_out=rs[h][:],
            )
            # bf16 x bf16 -> bf16 multiply: DVE 2x mode
            nc.vector.tensor_mul(
                em[:, h * 1024 : (h + 1) * 1024],
                ex[:, h * 1024 : (h + 1) * 1024],
                mt[:, h * 1024 : (h + 1) * 1024],
            )

            # previous q-tile's AV runs on PE between this tile's halves
            if pending is not None:
                emit_av(pending)
                pending = None

            # transpose this half's 8 [128,128] blocks; single 2x DVE copy out
            pst = ps_t.tile([P, 8 * P], BF16, tag="ps_t")
            for k in range(8):
                kt = 8 * h + k
                nc.tensor.transpose(
                    pst[:, k * P : (k + 1) * P],
                    em[:, kt * P : (kt + 1) * P],
                    identity_bf[:],
                )
            nc.vector.tensor_copy(
                emt[:, 8 * h : 8 * h + 8, :],
                pst[:].rearrange("p (a b) -> p a b", a=8),
            )

        rsum = rspool.tile([P, 1], F32, tag="rsum")
        nc.vector.tensor_add(rsum[:], rs[0][:], rs[1][:])
        recip = rspool.tile([P, 1], F32, tag="recip", name="recip")
        nc.vector.reciprocal(recip[:], rsum[:])

        pending = (qt, emt, recip)

    emit_av(pending)


def _build():
    # Bacc (not plain Bass): its compile() legalizes sync waits
    # (move_matmul_waits_to_ldweights + generate_event_semaphores) which
    # walrus codegen requires (max 1 wait per instruction).
    nc = bacc.Bacc("TRN2", target_bir_lowering=False, debug=False)
    io = {}
    for name in ("Q_gene", "K_gene", "Q_expr", "K_expr", "V_expr"):
        io[name] = nc.dram_tensor(name, [G, D], F32, kind="ExternalInput").ap()
    io["M"] = nc.dram_tensor("M", [G, G], F32, kind="ExternalInput").ap()
    for name in ("WK_w", "WQ_w"):
        io[name] = nc.dram_tensor(name, [D, 2 * D], F32, kind="ExternalInput").ap()
    for name in ("WK_b", "WQ_b"):
        io[name] = nc.dram_tensor(name, [D], F32, kind="ExternalInput").ap()
    io["out"] = nc.dram_tensor("out", [G, D], F32, kind="ExternalOutput").ap()

    with tile.TileContext(nc) as tc:
        with ExitStack() as ctx:
            _emit(ctx, tc, io)
    nc.compile()
    return nc


_NC = None


def _get_nc():
    global _NC
    if _NC is None:
        _NC = _build()
    return _NC


def kernel(**inputs) -> np.ndarray:
    return run_kernel_with_results(**inputs)[0]


def run_kernel_with_results(trace=False, **inputs):
    """Returns (full_output, BassKernelResults)."""
    nc = _get_nc()
    per_core_names = ("Q_gene", "K_gene", "Q_expr", "K_expr", "V_expr", "M")
    shared_names = ("WK_w", "WK_b", "WQ_w", "WQ_b")
    arrs = {k: np.ascontiguousarray(np.asarray(v), dtype=np.float32) for k, v in inputs.items()}
    in_maps = []
    for c in range(N_CORES):
        im = {n: arrs[n][c] for n in per_core_names}
        for n in shared_names:
            im[n] = arrs[n]
        in_maps.append(im)
    res = run_bass_kernel_spmd(nc, in_maps, list(range(N_CORES)), trace=trace)
    out = np.stack([res.results[c]["out"] for c in range(N_CORES)], axis=0)
    return out.astype(np.float32), res


# revision 7
# speedup vs baseline: 1.2425x; 1.2425x over previous
"""Bass/Tile TRN2 kernel for nn_ExpressionAttentionLayer.

Math per batch b (B=8, G=2048, D=64):
    K_fused = concat([K_gene, K_expr], -1) @ WK_w.T + WK_b      # (G, D)
    Q_fused = concat([Q_gene, Q_expr], -1) @ WQ_w.T + WQ_b      # (G, D)
    A       = softmax(Q_fused @ K_fused.T / sqrt(D), axis=-1)
    out     = (A * M) @ V_expr                                   # (G, D)

Sharding: data-parallel over batch; core i handles batch i (B == n_cores == 8).
No collectives.

Per-core dataflow (v3):
  - All Q/K loads use the "(p s)" partition-contiguous DRAM layout (1KB+
    descriptors).  PE transpose-mode runs directly on fp32 (1 pass on
    cayman), so no pre-casts.
  - K side feeds both HWDGE rings first (kfT gates every logits matmul);
    its transposed chunks scatter to stride-16 column views to restore
    natural k order (k must line up with M's columns).
  - Q side keeps the chunk order: loop iteration qt computes the 128
    logical q rows {p*16 + qt}.  The M tile and the output tile for
    iteration qt use row-permuted DRAM views (M stays 8KB/partition
    descriptors), so no on-chip fixup is needed and iteration qt only
    depends on Q chunk qt -> the loop starts ~7us in.
  - M streams in as bf16 via SWDGE cast-DMA half-tiles on the gpsimd
    queue (its own ring; 16 MB of the ~19.4 MB total HBM read traffic).
  - Per q-tile, per 1024-col half h: logits psum(fp32) = Q_tile.T @
    K_fusedT (bf16); exp on ScalarE -> bf16 ex with fp32 row-sum accum
    (scale=1/sqrt(D) folded in; |logits| <~ 7 so no max-subtraction
    needed); em = ex * M_bf16 on VectorE (all-bf16 -> 2x DVE mode);
    PE-transpose em (bf16 psum); single DVE 2x copy to [k, q] sbuf
    tiles; accumulate out over k-tiles on PE; apply the softmax
    reciprocal on ScalarE while copying out of PSUM; DMA out on sync.
"""

from contextlib import ExitStack

import numpy as np

import concourse.bass as bass
import concourse.tile as tile
from concourse import bacc, mybir
from concourse.bass_utils import run_bass_kernel_spmd
from concourse.masks import make_identity

B, G, D = 8, 2048, 64
P = 128
NT = G // P  # 16 tiles of 128 rows
F32 = mybir.dt.float32
BF16 = mybir.dt.bfloat16
AF = mybir.ActivationFunctionType

N_CORES = 8


def _emit(ctx: ExitStack, tc: tile.TileContext, io: dict):
    nc = tc.nc

    singles = ctx.enter_context(tc.tile_pool(name="singles", bufs=1))
    ld = ctx.enter_context(tc.tile_pool(name="ld", bufs=4))

    # PSUM pools (8 banks total: ps_l 2x2 + ps_t 2x1 + ps_o 2x1 = 8)
    ps_l = ctx.enter_context(tc.tile_pool(name="ps_l", bufs=2, space="PSUM"))
    ps_t = ctx.enter_context(tc.tile_pool(name="ps_t", bufs=2, space="PSUM"))
    ps_o = ctx.enter_context(tc.tile_pool(name="ps_o", bufs=2, space="PSUM"))

    identity = singles.tile([P, P], F32)
    make_identity(nc, identity[:])
    identity_bf = singles.tile([P, P], BF16)
    nc.vector.tensor_copy(identity_bf[:], identity[:])

    # ---- M-tile SWDGE cast-DMA lookahead queue (bf16 in SBUF) ----
    # Row-permuted view: iteration qt covers logical q rows {p*16 + qt},
    # matching the Q-side chunk order (see module docstring).  8KB
    # contiguous per partition per half -> line-rate descriptors.
    mpool = ctx.enter_context(tc.tile_pool(name="mpool", bufs=5))
    m_r = io["M"].rearrange("(p s) k -> s p k", s=NT)
    mts = {}

    def issue_m(qt):
        if qt < NT:
            mt = mpool.tile([P, G], BF16, tag="m", name="m")
            for h in range(2):
                nc.gpsimd.dma_start(
                    mt[:, h * 1024 : (h + 1) * 1024],
                    m_r[qt, :, h * 1024 : (h + 1) * 1024],
                )
            mts[qt] = mt

    for _qt in range(3):
        issue_m(_qt)

    # ---- HAM warmup bridge: ~6us of junk matmuls so the PE has no idle
    # window between kernel start and the first input transposes (a PE-idle
    # HAM window re-throttles the clock to 1.2 GHz and it can stick there).
    junk = singles.tile([P, 512], BF16, tag="junk")
    nc.vector.memset(junk[:], 0.0)
    for _ in range(14):
        psw = ps_o.tile([P, 512], F32, tag="ps_o", name="ps_warm")
        nc.tensor.matmul(psw[:], identity_bf[:], junk[:], start=True, stop=True)

    # ---- weights: WK_w/WQ_w are [D, 2D]; natural load, then PE-transpose the
    # two [64, 64] halves (base partition 0) and cast to bf16 lhsT tiles.
    wk_nat = singles.tile([D, 2 * D], F32, tag="wk_nat")
    wq_nat = singles.tile([D, 2 * D], F32, tag="wq_nat")
    nc.sync.dma_start(wk_nat[:], io["WK_w"][:, :])
    nc.scalar.dma_start(wq_nat[:], io["WQ_w"][:, :])
    wk_gTb = singles.tile([D, D], BF16, tag="wk_gTb")
    wk_eTb = singles.tile([D, D], BF16, tag="wk_eTb")
    wq_gTb = singles.tile([D, D], BF16, tag="wq_gTb")
    wq_eTb = singles.tile([D, D], BF16, tag="wq_eTb")
    for nat, dsts in ((wk_nat, (wk_gTb, wk_eTb)), (wq_nat, (wq_gTb, wq_eTb))):
        for h, dst in enumerate(dsts):
            psw = ps_o.tile([P, 512], F32, tag="ps_o", name="ps_w")
            nc.tensor.transpose(
                psw[:D, :D], nat[:, h * D : (h + 1) * D], identity[:D, :D]
            )
            nc.vector.tensor_copy(dst[:], psw[:D, :D])
    wkb = singles.tile([D, 1], F32, tag="wkb")
    wqb = singles.tile([D, 1], F32, tag="wqb")
    nc.sync.dma_start(wkb[:], io["WK_b"][:, None])
    nc.scalar.dma_start(wqb[:], io["WQ_b"][:, None])

    # ---- input loads: "(p s)" partition-contiguous layout, quartered.
    # K side split across BOTH HWDGE rings (it gates everything); Q side
    # follows on both rings; V last (first needed by the AV stage).
    bigs = {}
    for src_name, eng in (
        ("K_gene", nc.sync),
        ("K_expr", nc.scalar),
        ("Q_gene", nc.sync),
        ("Q_expr", nc.scalar),
    ):
        big = ld.tile([P, NT, D], F32, tag=f"ld_{src_name}", name=f"ld_{src_name}")
        r = io[src_name].rearrange("(p s) d -> p s d", s=NT)
        for q4 in range(4):
            eng.dma_start(big[:, 4 * q4 : 4 * q4 + 4, :], r[:, 4 * q4 : 4 * q4 + 4, :])
        bigs[src_name] = big

    # bf16 pre-casts (per quarter) so the PE transposes are 1-pass bf16;
    # K side on DVE, Q side on ACT — both near-idle in the prologue.
    bigs_bf = {}
    for src_name in ("K_gene", "K_expr", "Q_gene", "Q_expr"):
        big_bf = ld.tile([P, NT, D], BF16, tag=f"ldb_{src_name}", name=f"ldb_{src_name}")
        for q4 in range(4):
            sl = (slice(None), slice(4 * q4, 4 * q4 + 4), slice(None))
            if src_name.startswith("K"):
                nc.vector.tensor_copy(big_bf[sl], bigs[src_name][sl])
            else:
                nc.scalar.copy(big_bf[sl], bigs[src_name][sl])
        bigs_bf[src_name] = big_bf

    # ---- V in [128, NT, D] (natural g on partitions; 256B descriptors but
    # only 512KB and not needed until the first AV), cast to bf16.
    v_sb = singles.tile([P, NT, D], F32, tag="v")
    nc.scalar.dma_start(v_sb[:], io["V_expr"].rearrange("(t p) d -> p t d", p=P))
    v_bf = singles.tile([P, NT, D], BF16, tag="v_bf")
    nc.vector.tensor_copy(v_bf[:], v_sb[:])

    # ---- transpose K/Q gene+expr into bf16 [D, G] (d on partitions) ----
    # Chunk s of the "(p s)" load transposes to columns {p*16 + s}.  K side
    # scatters through a stride-16 view to restore natural k order; Q side
    # keeps chunk order (the q permutation is absorbed by the M/out views).
    kgT = singles.tile([D, G], BF16, tag="kgT")
    keT = singles.tile([D, G], BF16, tag="keT")
    qgT = singles.tile([D, G], BF16, tag="qgT")
    qeT = singles.tile([D, G], BF16, tag="qeT")
    kfT = singles.tile([D, G], BF16, tag="kfT")
    qfT = singles.tile([D, G], BF16, tag="qfT")

    def emit_transposes(side, gT, eT, wgT, weT, b_sb, fT, j):
        for c, dstT, ceng in ((0, gT, nc.vector), (1, eT, nc.scalar)):
            big = bigs_bf[f"{side}_gene" if c == 0 else f"{side}_expr"]
            ps = ps_t.tile([P, 8 * P], BF16, tag="ps_t", name="ps_tr")[:D, : 4 * P]
            for i in range(4):
                s = 4 * j + i
                nc.tensor.transpose(
                    ps[:, i * P : (i + 1) * P], big[:, s, :], identity_bf[:]
                )
            src = ps[:].rearrange("d (i p) -> d i p", i=4)
            if side == "K":
                # natural order: column p*16 + s <- chunk s position p
                dst = dstT[:].rearrange("d (p s) -> d s p", s=NT)[:, 4 * j : 4 * j + 4, :]
            else:
                # chunk order: chunk s occupies columns [s*128, (s+1)*128)
                dst = dstT[:, j * 512 : (j + 1) * 512].rearrange(
                    "d (i p) -> d i p", i=4
                )
            if c == 0:
                ceng.tensor_copy(dst, src)
            else:
                ceng.copy(dst, src)
        if side == "Q":
            # Q projections can run per block (block j = chunks 4j..4j+3)
            emit_proj(gT, eT, wgT, weT, b_sb, fT, j)

    def emit_proj(gT, eT, wgT, weT, b_sb, fT, j):
        psj = ps_o.tile([P, 512], F32, tag="ps_o", name="ps_pj")[:D, :]
        nc.tensor.matmul(
            psj[:], wgT[:], gT[:, j * 512 : (j + 1) * 512], start=True, stop=False
        )
        nc.tensor.matmul(
            psj[:], weT[:], eT[:, j * 512 : (j + 1) * 512], start=False, stop=True
        )
        nc.scalar.activation(
            fT[:, j * 512 : (j + 1) * 512], psj[:], AF.Identity, bias=b_sb[:, 0:1]
        )

    for j in range(4):
        emit_transposes("K", kgT, keT, wk_gTb, wk_eTb, wkb, kfT, j)
    for j in range(4):
        # K projections only after all K chunks scattered (stride-16 mix)
        emit_proj(kgT, keT, wk_gTb, wk_eTb, wkb, kfT, j)
    for j in range(4):
        emit_transposes("Q", qgT, qeT, wq_gTb, wq_eTb, wqb, qfT, j)

    # ---- main attention loop (fully per-q-tile pipelined) ----
    epool = ctx.enter_context(tc.tile_pool(name="epool", bufs=2))
    empool = ctx.enter_context(tc.tile_pool(name="empool", bufs=2))
    tpool = ctx.enter_context(tc.tile_pool(name="tpool", bufs=2))
    opool = ctx.enter_context(tc.tile_pool(name="opool", bufs=2))
    rspool = ctx.enter_context(tc.tile_pool(name="rspool", bufs=4))

    out_r = io["out"].rearrange("(p s) d -> s p d", s=NT)
    scale = 1.0 / np.sqrt(np.float32(D))

    # Per-qt state carried one step so the AV matmuls of qt-1 are emitted
    # between qt's logits and qt's transposes — PE chews on them while the
    # ScalarE/VectorE stages of qt run, instead of stalling at a group
    # barrier.
    pending = None  # (qt, emt, recip)

    def emit_av(pend):
        qt_p, emt_p, recip_p = pend
        # out[q, d] += expM^T_chunk.T @ V  (lhsT=emt chunk: 128 bf16 cols -> FWL)
        pso = ps_o.tile([P, 512], F32, tag="ps_o", name="ps_av")[:, :D]
        for kt in range(NT):
            nc.tensor.matmul(
                pso[:],
                emt_p[:, kt, :],
                v_bf[:, kt, :],
                start=(kt == 0),
                stop=(kt == NT - 1),
            )
        ob = opool.tile([P, D], F32, tag="ob")
        # apply softmax denominator while copying out of PSUM
        nc.scalar.activation(ob[:], pso[:], AF.Copy, bias=0.0, scale=recip_p[:, 0:1])
        nc.sync.dma_start(out_r[qt_p], ob[:])

    for qt in range(NT):
        mt = mts.pop(qt)
        issue_m(qt + 3)

        ex = epool.tile([P, G], BF16, tag="ex")
        em = empool.tile([P, G], BF16, tag="em")
        emt = tpool.tile([P, NT, P], BF16, tag="emt")  # expM^T tiles [k, q]
        rs = [rspool.tile([P, 1], F32, tag=f"rs{h}", name=f"rs{h}") for h in range(2)]

        for h in range(2):
            # logits in a [128, 1024] psum tile (2 banks)
            psl = ps_l.tile([P, 1024], F32, tag="ps_l")
            for n in range(2):
                nc.tensor.matmul(
                    psl[:, n * 512 : (n + 1) * 512],
                    qfT[:, qt * P : (qt + 1) * P],
                    kfT[:, (2 * h + n) * 512 : (2 * h + n + 1) * 512],
                    start=True,
                    stop=True,
                )
            # exp -> bf16 with fp32 row-sum accumulation
            nc.scalar.activation(
                ex[:, h * 1024 : (h + 1) * 1024],
                psl[:],
                AF.Exp,
                scale=float(scale),
                accum_out=rs[h][:],
            )
            # bf16 x bf16 -> bf16 multiply: DVE 2x mode
            nc.vector.tensor_mul(
                em[:, h * 1024 : (h + 1) * 1024],
                ex[:, h * 1024 : (h + 1) * 1024],
                mt[:, h * 1024 : (h + 1) * 1024],
            )

            # previous q-tile's AV runs on PE between this tile's halves
            if pending is not None:
                emit_av(pending)
                pending = None

            # transpose this half's 8 [128,128] blocks; single 2x DVE copy out
            pst = ps_t.tile([P, 8 * P], BF16, tag="ps_t")
            for k in range(8):
                kt = 8 * h + k
                nc.tensor.transpose(
                    pst[:, k * P : (k + 1) * P],
                    em[:, kt * P : (kt + 1) * P],
                    identity_bf[:],
                )
            nc.vector.tensor_copy(
                emt[:, 8 * h : 8 * h + 8, :],
                pst[:].rearrange("p (a b) -> p a b", a=8),
            )

        rsum = rspool.tile([P, 1], F32, tag="rsum")
        nc.vector.tensor_add(rsum[:], rs[0][:], rs[1][:])
        recip = rspool.tile([P, 1], F32, tag="recip", name="recip")
        nc.vector.reciprocal(recip[:], rsum[:])

        pending = (qt, emt, recip)

    emit_av(pending)


def _build():
    # Bacc (not plain Bass): its compile() legalizes sync waits
    # (move_matmul_waits_to_ldweights + generate_event_semaphores) which
    # walrus codegen requires (max 1 wait per instruction).
    nc = bacc.Bacc("TRN2", target_bir_lowering=False, debug=False)
    io = {}
    for name in ("Q_gene", "K_gene", "Q_expr", "K_expr", "V_expr"):
        io[name] = nc.dram_tensor(name, [G, D], F32, kind="ExternalInput").ap()
    io["M"] = nc.dram_tensor("M", [G, G], F32, kind="ExternalInput").ap()
    for name in ("WK_w", "WQ_w"):
        io[name] = nc.dram_tensor(name, [D, 2 * D], F32, kind="ExternalInput").ap()
    for name in ("WK_b", "WQ_b"):
        io[name] = nc.dram_tensor(name, [D], F32, kind="ExternalInput").ap()
    io["out"] = nc.dram_tensor("out", [G, D], F32, kind="ExternalOutput").ap()

    with tile.TileContext(nc) as tc:
        with ExitStack() as ctx:
            _emit(ctx, tc, io)
    nc.compile()
    return nc


_NC = None


def _get_nc():
    global _NC
    if _NC is None:
        _NC = _build()
    return _NC


def kernel(**inputs) -> np.ndarray:
    return run_kernel_with_results(**inputs)[0]


def run_kernel_with_results(trace=False, **inputs):
    """Returns (full_output, BassKernelResults)."""
    nc = _get_nc()
    per_core_names = ("Q_gene", "K_gene", "Q_expr", "K_expr", "V_expr", "M")
    shared_names = ("WK_w", "WK_b", "WQ_w", "WQ_b")
    arrs = {k: np.ascontiguousarray(np.asarray(v), dtype=np.float32) for k, v in inputs.items()}
    in_maps = []
    for c in range(N_CORES):
        im = {n: arrs[n][c] for n in per_core_names}
        for n in shared_names:
            im[n] = arrs[n]
        in_maps.append(im)
    res = run_bass_kernel_spmd(nc, in_maps, list(range(N_CORES)), trace=trace)
    out = np.stack([res.results[c]["out"] for c in range(N_CORES)], axis=0)
    return out.astype(np.float32), res
